# revision 1
# baseline (speedup 1.0000x reference)
"""Trainium2 Bass kernel for nn_CausalSelfAttention (erf-kernel attention).

Sharding: 8 cores = 2 batches x 4 core-groups; each core handles one batch
and 3 of the 12 heads (data-parallel over batch, head-parallel within batch).
Each core computes its 3 heads' full attention plus its partial output
projection; the host sums the 4 partials per batch.

Device-side layout strategy (per core):
  - x arrives pre-transposed from host: xT [768, 2048] (contract dim on
    partitions for the QKV matmuls), in the matmul storage dtype.
  - Host packs/permutes QKV weight rows into one [768, 576] matrix "wall"
    whose 5 output chunks of <=128 land directly in the SBUF row layout the
    rest of the kernel wants:
        C1 = [q_h0 | q_h1]   (rope-permuted rows: even dims then odd dims)
        C2 = [k_h0 | k_h1]
        C3 = [v_h0 | q_h2]
        C4 = [v_h1 | k_h2]
        C5 = [v_h2]
    The even/odd permutation makes RoPE operate on contiguous 32-partition
    blocks; scores are invariant to a shared q/k head-dim permutation.
  - RoPE: partner swap via a PE permutation matmul, then 3 DVE ops in fp32,
    writing rotated q/k into fresh tiles of the matmul dtype.
  - Scores computed transposed: sT[t, s] = kT.T @ qT per (128 t x 512 s)
    tile, causal tiles only.  erf(0.125*x) on ACT, +1 on DVE, diagonal
    band masked with affine_select on Pool.
  - AV: yT[d, s] accumulated in PSUM over t-chunks with v_ext [t, 65]
    stationary (65th column of ones produces the softmax-denominator row).
  - Normalization: reciprocal of denom row, replicated across partitions by
    a K=1 matmul, multiplied into yT.
  - Projection: out[s, e] = yT.T @ WprojT per head, PSUM-accumulated across
    heads, copied to SBUF and DMA'd to DRAM.

Matmul storage dtype (KERNEL_DTYPE): fp32 (4 cyc/row, exact), f32r
(1 cyc/row at N>=256, reduced mantissa), bf16 (1 cyc/row, 2-byte storage).
PSUM accumulation is always fp32.
"""

import os
import sys
from contextlib import ExitStack

import numpy as np

for _p in ("/opt/trn_rl_repo",):
    if _p not in sys.path:
        sys.path.insert(0, _p)

import concourse.bass as bass
import concourse.mybir as mybir
from concourse.bass_utils import run_bass_kernel_spmd
from concourse.tile import TileContext

S = 2048          # sequence length per batch
D = 768           # model dim
HD = 64           # head dim
HPC = 3           # heads per core
NCORES = 8
F32 = mybir.dt.float32
NT = S // 512     # 4 free-dim tiles of 512
TC = S // 128     # 16 t-chunks of 128
EPS = 1e-6

DTYPE_NAME = os.environ.get("KERNEL_DTYPE", "f32r")
IN_DT = {
    "fp32": mybir.dt.float32,
    "f32r": mybir.dt.float32r,
    "bf16": mybir.dt.bfloat16,
}[DTYPE_NAME]
# CoreSim doesn't implement Erf; dev-only switch to validate logic in sim.
ERF_FUNC_NAME = "Tanh" if os.environ.get("KERNEL_SIM_TANH", "0") == "1" else "Erf"

LAST_EXEC_NS = None
LAST_RESULTS = None


def _split_multi_waits(nc: bass.Bass) -> None:
    """This walrus build rejects instructions carrying more than one sync
    wait (codegen 'Too many sync wait commands', hit by the Tile kernel-tail
    drain).  Hoist all but the last wait of any multi-wait instruction onto
    single-wait Drain instructions inserted just before it on the same
    engine — semantically identical, one wait per instruction."""
    for f in nc.m.functions:
        for b in f.blocks:
            new_insts = []
            changed = False
            for inst in b.instructions:
                si = inst.sync_info
                waits = list(si.on_wait) if si is not None and si.on_wait else []
                if len(waits) > 1:
                    changed = True
                    for n, w in enumerate(waits[:-1]):
                        d = mybir.InstDrain(
                            name=f"{inst.name}-wsplit{n}",
                            engine=inst.engine,
                            ins=[],
                            outs=[],
                            sync_info=mybir.SyncInfo(on_wait=[w], on_update=[]),
                        )
                        new_insts.append(d)
                    si.on_wait = [waits[-1]]
                new_insts.append(inst)
            if changed:
                b.instructions[:] = new_insts


def build_program() -> bass.Bass:
    nc = bass.Bass(target_bir_lowering=False, debug=False)

    x_t = nc.declare_dram_parameter("xt", [D, S], IN_DT, isOutput=False)
    wall = nc.declare_dram_parameter("wall", [D, 576], IN_DT, isOutput=False)
    wproj = nc.declare_dram_parameter("wproj", [HPC * HD, D], IN_DT, isOutput=False)
    csc = nc.declare_dram_parameter("csc", [128, S], F32, isOutput=False)
    css = nc.declare_dram_parameter("css", [128, S], F32, isOutput=False)
    swp = nc.declare_dram_parameter("swp", [128, 128], IN_DT, isOutput=False)
    iden = nc.declare_dram_parameter("iden", [128, 128], F32, isOutput=False)
    out_d = nc.declare_dram_parameter("out", [S, D], F32, isOutput=True)

    with TileContext(nc) as tc:
        with ExitStack() as ctx:
            const = ctx.enter_context(tc.tile_pool(name="const", bufs=1))
            big = ctx.enter_context(tc.tile_pool(name="big", bufs=10))
            wpool = ctx.enter_context(tc.tile_pool(name="wpool", bufs=3))
            tpool = ctx.enter_context(tc.tile_pool(name="tpool", bufs=2))
            npool = ctx.enter_context(tc.tile_pool(name="npool", bufs=2))
            ps_a = ctx.enter_context(tc.tile_pool(name="ps_a", bufs=3, space="PSUM"))
            ps_s = ctx.enter_context(tc.tile_pool(name="ps_s", bufs=2, space="PSUM"))
            ps_y = ctx.enter_context(tc.tile_pool(name="ps_y", bufs=2, space="PSUM"))
            ps_r = ctx.enter_context(tc.tile_pool(name="ps_r", bufs=1, space="PSUM"))

            # ---- constants / inputs ----
            XT = []
            for kc in range(6):
                t = big.tile([128, S], IN_DT, tag="big", name=f"xt{kc}")
                nc.sync.dma_start(out=t, in_=x_t[kc * 128:(kc + 1) * 128, :])
                XT.append(t)
            WA = []
            for kc in range(6):
                t = const.tile([128, 576], IN_DT, tag=f"wa{kc}", name=f"wa{kc}")
                nc.sync.dma_start(out=t, in_=wall[kc * 128:(kc + 1) * 128, :])
                WA.append(t)
            WP = []
            for h in range(HPC):
                t = const.tile([HD, D], IN_DT, tag=f"wp{h}", name=f"wp{h}")
                nc.sync.dma_start(out=t, in_=wproj[h * HD:(h + 1) * HD, :])
                WP.append(t)
            CSC = const.tile([128, S], F32, tag="csc")
            nc.sync.dma_start(out=CSC, in_=csc[:, :])
            CSS = const.tile([128, S], F32, tag="css")
            nc.sync.dma_start(out=CSS, in_=css[:, :])
            SWP = const.tile([128, 128], IN_DT, tag="swp")
            nc.sync.dma_start(out=SWP, in_=swp[:, :])
            ID = const.tile([128, 128], F32, tag="iden")
            nc.sync.dma_start(out=ID, in_=iden[:, :])
            ONESF = const.tile([128, HD], F32, tag="onesf")
            nc.vector.memset(ONESF, 1.0)
            ONES = const.tile([128, HD], IN_DT, tag="ones")
            nc.vector.tensor_copy(out=ONES, in_=ONESF)

            # ---- QKV projection: packed q/k/v row chunks ----
            # C1, C2 (pure q/k) rotate through the big pool — freed after
            # RoPE.  C3, C4, C5 carry v rows for the whole kernel -> const.
            C1 = big.tile([128, S], F32, tag="big", name="c1")
            C2 = big.tile([128, S], F32, tag="big", name="c2")
            C3 = const.tile([128, S], F32, tag="c3")
            C4 = const.tile([128, S], F32, tag="c4")
            C5 = const.tile([64, S], F32, tag="c5")
            RAW = [C1, C2, C3, C4, C5]
            for m in range(5):
                msz = 128 if m < 4 else 64
                for nt in range(NT):
                    ns = slice(nt * 512, (nt + 1) * 512)
                    ps = ps_a.tile([128, 512], F32, tag="ps_a")
                    for kc in range(6):
                        nc.tensor.matmul(
                            ps[:msz, :],
                            lhsT=WA[kc][:, m * 128:m * 128 + msz],
                            rhs=XT[kc][:, ns],
                            start=(kc == 0),
                            stop=(kc == 5),
                        )
                    nc.vector.tensor_copy(out=RAW[m][:, ns], in_=ps[:msz, :])

            # ---- RoPE: rotate q/k rows into fresh IN_DT tiles ----
            # rows [r0, r0+64) hold one head's [even dims | odd dims]; the
            # partner value (odd for even rows, even for odd rows) comes from
            # a PE swap matmul; csc/css carry cos and sign-baked sin.
            QF = big.tile([128, S], IN_DT, tag="big", name="qf")
            KF = big.tile([128, S], IN_DT, tag="big", name="kf")
            Q2F = big.tile([128, S], IN_DT, tag="big", name="q2f")
            K2F = big.tile([128, S], IN_DT, tag="big", name="k2f")

            def rope(raw, out, r0, rsz):
                # The swap matmul always runs on all 128 rows with output at
                # partition 0 (f32r matmuls require dst partition 0; extra
                # rows cost nothing — matmul time is column count).  The
                # rotate ops then slice the rows they need, base-aligned.
                nrows = raw.shape[0]
                for nt in range(NT):
                    ns = slice(nt * 512, (nt + 1) * 512)
                    rs = slice(r0, r0 + rsz)
                    if IN_DT != F32:
                        # swap-matmul needs an IN_DT rhs produced by a
                        # rounding op (the BIR verifier rejects bitcasts
                        # into f32r): convert via a copy first
                        swin = tpool.tile([128, 512], IN_DT, tag="cv")
                        nc.vector.tensor_copy(out=swin[:nrows, :], in_=raw[:, ns])
                        swin_ap = swin[:nrows, :]
                    else:
                        swin_ap = raw[:, ns]
                    sw = ps_a.tile([128, 512], F32, tag="ps_a")
                    nc.tensor.matmul(
                        sw[:nrows, :],
                        lhsT=SWP[:nrows, :nrows],
                        rhs=swin_ap,
                        start=True,
                        stop=True,
                    )
                    t1 = tpool.tile([128, 512], F32, tag="t1")
                    t2 = tpool.tile([128, 512], F32, tag="t2")
                    nc.vector.tensor_mul(t1[rs, :], raw[rs, ns], CSC[rs, ns])
                    nc.vector.tensor_mul(t2[rs, :], sw[rs, :], CSS[rs, ns])
                    nc.vector.tensor_add(out[rs, ns], t1[rs, :], t2[rs, :])

            rope(C1, QF, 0, 128)     # q_h0, q_h1
            rope(C2, KF, 0, 128)     # k_h0, k_h1
            rope(C3, Q2F, 64, 64)    # q_h2 (rows 64:128; rows 0:64 are v_h0)
            rope(C4, K2F, 64, 64)    # k_h2

            # ---- v_ext[h]: 16 chunks of [128 t, 65] = [v^T chunk | ones] ----
            VSRC = [C3[0:64, :], C4[0:64, :], C5[0:64, :]]
            VEXT = []
            for h in range(HPC):
                ve = big.tile([128, TC * 65], IN_DT, tag="big", name=f"vext{h}")
                VEXT.append(ve)
            for h in range(HPC):
                # ones into every 65th column (the denominator generator)
                ve3 = VEXT[h].rearrange("p (t c) -> p t c", c=65)
                nc.vector.tensor_copy(out=ve3[:, :, 64], in_=ONESF[:, 0:TC])
                for tcb in range(TC):
                    pt = ps_a.tile([128, 512], F32, tag="ps_a")
                    nc.tensor.transpose(
                        pt[:, 0:HD],
                        in_=VSRC[h][:, tcb * 128:(tcb + 1) * 128],
                        identity=ID[0:HD, 0:HD],
                    )
                    nc.vector.tensor_copy(
                        out=VEXT[h][:, tcb * 65:tcb * 65 + HD], in_=pt[:, 0:HD]
                    )

            # ---- attention per head ----
            QSRC = [QF[0:64, :], QF[64:128, :], Q2F[64:128, :]]
            KSRC = [KF[0:64, :], KF[64:128, :], K2F[64:128, :]]
            YT = []
            for h in range(HPC):
                YT.append(big.tile([HD, S], IN_DT, tag="big", name=f"yt{h}"))

            for h in range(HPC):
                q, k = QSRC[h], KSRC[h]
                for si in range(NT):
                    ss = slice(si * 512, (si + 1) * 512)
                    ntc = 4 * (si + 1)
                    yps = ps_y.tile([65, 512], F32, tag="ps_y")
                    for tcb in range(ntc):
                        sc = ps_s.tile([128, 512], F32, tag="ps_s")
                        nc.tensor.matmul(
                            sc,
                            lhsT=k[:, tcb * 128:(tcb + 1) * 128],
                            rhs=q[:, ss],
                            start=True,
                            stop=True,
                        )
                        wt = wpool.tile([128, 512], IN_DT, tag="wt")
                        nc.scalar.activation(
                            out=wt, in_=sc,
                            func=getattr(mybir.ActivationFunctionType, ERF_FUNC_NAME),
                            scale=0.125,
                        )
                        nc.vector.tensor_scalar_add(wt, wt, 1.0)
                        if tcb >= 4 * si:
                            # diagonal band: zero the t > s corner
                            nc.gpsimd.affine_select(
                                out=wt, in_=wt,
                                compare_op=mybir.AluOpType.is_ge,
                                fill=0.0,
                                base=si * 512 - tcb * 128,
                                channel_multiplier=-1,
                                pattern=[[1, 512]],
                            )
                        nc.tensor.matmul(
                            yps,
                            lhsT=VEXT[h][:, tcb * 65:(tcb + 1) * 65],
                            rhs=wt,
                            start=(tcb == 0),
                            stop=(tcb == ntc - 1),
                        )
                    # normalize: yT[0:64] / max(denom row, eps)
                    dmx = npool.tile([65, 512], F32, tag="dmx")
                    nc.vector.tensor_scalar_max(dmx[64:65, :], yps[64:65, :], EPS)
                    rcpf = npool.tile([65, 512], F32, tag="rcpf")
                    nc.vector.reciprocal(rcpf[64:65, :], dmx[64:65, :])
                    rcp = npool.tile([65, 512], IN_DT, tag="rcp")
                    nc.vector.tensor_copy(out=rcp[64:65, :], in_=rcpf[64:65, :])
                    rep = ps_r.tile([HD, 512], F32, tag="ps_r")
                    nc.tensor.matmul(
                        rep,
                        lhsT=ONES[64:65, 0:HD],
                        rhs=rcp[64:65, :],
                        start=True,
                        stop=True,
                    )
                    rsb = npool.tile([HD, 512], F32, tag="rsb")
                    nc.vector.tensor_copy(out=rsb, in_=rep)
                    nc.vector.tensor_mul(YT[h][:, ss], yps[0:64, :], rsb)

            # ---- output projection (partial over this core's heads) ----
            for sci in range(TC):
                scs = slice(sci * 128, (sci + 1) * 128)
                po1 = ps_a.tile([128, 512], F32, tag="ps_a")
                po2 = ps_a.tile([128, 512], F32, tag="ps_a")
                for h in range(HPC):
                    nc.tensor.matmul(
                        po1,
                        lhsT=YT[h][:, scs],
                        rhs=WP[h][:, 0:512],
                        start=(h == 0),
                        stop=(h == HPC - 1),
                    )
                    nc.tensor.matmul(
                        po2[:, 0:256],
                        lhsT=YT[h][:, scs],
                        rhs=WP[h][:, 512:768],
                        start=(h == 0),
                        stop=(h == HPC - 1),
                    )
                ost = tpool.tile([128, D], F32, tag="ost", bufs=3)
                if sci % 2 == 0:
                    nc.scalar.copy(out=ost[:, 0:512], in_=po1)
                    nc.vector.tensor_copy(out=ost[:, 512:768], in_=po2[:, 0:256])
                else:
                    nc.vector.tensor_copy(out=ost[:, 0:512], in_=po1)
                    nc.scalar.copy(out=ost[:, 512:768], in_=po2[:, 0:256])
                nc.sync.dma_start(out=out_d[scs, :], in_=ost)

    return nc


_PROGRAM = None


def _get_program() -> bass.Bass:
    global _PROGRAM
    if _PROGRAM is None:
        _PROGRAM = build_program()
        _split_multi_waits(_PROGRAM)
    return _PROGRAM


def _np_indt(arr):
    return np.ascontiguousarray(arr).astype(mybir.dt.np(IN_DT))


def make_in_maps(x, Wq, Wk, Wv, Wproj):
    x = np.asarray(x, dtype=np.float32)
    Wq = np.asarray(Wq, dtype=np.float32)
    Wk = np.asarray(Wk, dtype=np.float32)
    Wv = np.asarray(Wv, dtype=np.float32)
    Wproj = np.asarray(Wproj, dtype=np.float32)

    half = HD // 2
    j = np.arange(half, dtype=np.float64)
    freq = 1.0 / (10000.0 ** (j / half))
    ang = np.arange(S, dtype=np.float64)[None, :] * freq[:, None]   # [32, S]
    cosT = np.cos(ang).astype(np.float32)
    sinT = np.sin(ang).astype(np.float32)
    csc = np.tile(np.vstack([cosT, cosT]), (2, 1))                  # [128, S]
    css = np.tile(np.vstack([-sinT, sinT]), (2, 1))

    swp = np.zeros((128, 128), dtype=np.float32)
    for blk in range(2):
        for jj in range(half):
            swp[blk * 64 + jj, blk * 64 + half + jj] = 1.0
            swp[blk * 64 + half + jj, blk * 64 + jj] = 1.0
    iden = np.eye(128, dtype=np.float32)

    perm = np.concatenate([np.arange(0, HD, 2), np.arange(1, HD, 2)])

    in_maps = []
    for c in range(NCORES):
        b = c // 4
        hs = [(c % 4) * HPC + i for i in range(HPC)]
        rq = [Wq[h * HD:(h + 1) * HD][perm, :] for h in hs]
        rk = [Wk[h * HD:(h + 1) * HD][perm, :] for h in hs]
        rv = [Wv[h * HD:(h + 1) * HD, :] for h in hs]
        cols = np.concatenate(
            [rq[0], rq[1], rk[0], rk[1], rv[0], rq[2], rv[1], rk[2], rv[2]],
            axis=0,
        )                                                           # [576, D]
        wall = np.ascontiguousarray(cols.T)                         # [D, 576]
        dims = np.concatenate([np.arange(h * HD, (h + 1) * HD) for h in hs])
        wproj_t = np.ascontiguousarray(Wproj[:, dims].T)            # [192, D]
        in_maps.append({
            "xt": _np_indt(x[b].T),
            "wall": _np_indt(wall),
            "wproj": _np_indt(wproj_t),
            "csc": csc,
            "css": css,
            "swp": _np_indt(swp),
            "iden": iden,
        })
    return in_maps


def kernel(x, Wq, Wk, Wv, Wproj):
    global LAST_EXEC_NS, LAST_RESULTS
    nc = _get_program()
    in_maps = make_in_maps(x, Wq, Wk, Wv, Wproj)
    trace = os.environ.get("KERNEL_TRACE", "0") == "1"
    res = run_bass_kernel_spmd(nc, in_maps, list(range(NCORES)), trace=trace)
    LAST_EXEC_NS = res.exec_time_ns
    LAST_RESULTS = res
    outs = [np.asarray(r["out"], dtype=np.float32) for r in res.results]
    out = np.empty((2, S, D), dtype=np.float32)
    out[0] = outs[0] + outs[1] + outs[2] + outs[3]
    out[1] = outs[4] + outs[5] + outs[6] + outs[7]
    return out



# revision 29
# speedup vs baseline: 1.2477x; 1.2477x over previous
"""Trainium2 Bass kernel for nn_CausalSelfAttention (erf-kernel attention).

Sharding: 8 cores = 2 batches x 4 core-groups; each core handles one batch
and 3 of the 12 heads (data-parallel over batch, head-parallel within batch).
Each core computes its 3 heads' full attention plus its partial output
projection; the host sums the 4 partials per batch.

Device-side layout strategy (per core), all matmul inputs in bf16:
  - x arrives pre-transposed from host: xT [768, 2048] bf16.
  - Q/K weight "wall" [768, 384]: chunks C1=[q0|q1], C2=[k0|k1], C3=[q2|k2],
    each head's rows rope-permuted ([even dims | odd dims]) so RoPE operates
    on contiguous 32-partition blocks.
  - v^T computed directly: per 128-t-chunk, psum[128,192] = xT[:,tch].T@WvT,
    scattered into vall [128, 3*16*68]: per (head, chunk) 68 cols =
    [v^T (64) | one-hot ones column at 64+h | pad].  The ones column makes
    the AV matmul emit that head's softmax denominator at psum row 64+h, a
    distinct partition per head so denominators batch across heads.
  - RoPE: partner swap via PE permutation matmul, cos/sin multiplies on DVE
    in bf16 (2x perf mode), swap output staged through ACT copy.
  - Scores transposed: sT[t,s] = kT.T @ qT per (128t x 512s) causal tile,
    erf(0.125*x) on ACT -> bf16, +1 on DVE (4x mode), diagonal band masked
    with affine_select on Pool.
  - AV: yT[d,s] accumulated in PSUM over t-chunks, M=68 (64 dims + one-hot
    denominator rows).  Unnormalized yT copied to SBUF; denominators for the
    3 heads land on partitions 64..66 and are reciprocal-approximated in one
    batched DVE op per si, broadcast via a K=3 matmul with one-hot E3.
  - Projection: heads K-stacked (YT01 [128,S] + YT2 [64,S]): 2 accumulating
    matmuls per output half instead of 3.
"""

import os
import sys
from contextlib import ExitStack

import numpy as np

for _p in ("/opt/trn_rl_repo",):
    if _p not in sys.path:
        sys.path.insert(0, _p)

import concourse.bass as bass
import concourse.mybir as mybir
from concourse.bass_utils import run_bass_kernel_spmd
from concourse.tile import TileContext

S = 2048          # sequence length per batch
D = 768           # model dim
HD = 64           # head dim
HPC = 3           # heads per core
NCORES = 8
F32 = mybir.dt.float32
NT = S // 512     # 4 free-dim tiles of 512
TC = S // 128     # 16 t-chunks of 128
VW = 66           # vall chunk width: 64 v dims + ones col + pad

DTYPE_NAME = os.environ.get("KERNEL_DTYPE", "bf16")
IN_DT = {
    "fp32": mybir.dt.float32,
    "f32r": mybir.dt.float32r,
    "bf16": mybir.dt.bfloat16,
}[DTYPE_NAME]
# CoreSim doesn't implement Erf; dev-only switch to validate logic in sim.
ERF_FUNC_NAME = "Tanh" if os.environ.get("KERNEL_SIM_TANH", "0") == "1" else "Erf"

LAST_EXEC_NS = None
LAST_RESULTS = None


def _split_multi_waits(nc: bass.Bass) -> None:
    """This walrus build rejects instructions carrying more than one sync
    wait (codegen 'Too many sync wait commands', hit by the Tile kernel-tail
    drain).  Hoist all but the last wait of any multi-wait instruction onto
    single-wait Drain instructions inserted just before it on the same
    engine — semantically identical, one wait per instruction."""
    for f in nc.m.functions:
        for b in f.blocks:
            new_insts = []
            changed = False
            for inst in b.instructions:
                si = inst.sync_info
                waits = list(si.on_wait) if si is not None and si.on_wait else []
                if len(waits) > 1:
                    changed = True
                    for n, w in enumerate(waits[:-1]):
                        d = mybir.InstDrain(
                            name=f"{inst.name}-wsplit{n}",
                            engine=inst.engine,
                            ins=[],
                            outs=[],
                            sync_info=mybir.SyncInfo(on_wait=[w], on_update=[]),
                        )
                        new_insts.append(d)
                    si.on_wait = [waits[-1]]
                new_insts.append(inst)
            if changed:
                b.instructions[:] = new_insts


def build_program() -> bass.Bass:
    nc = bass.Bass(target_bir_lowering=False, debug=False)

    x_t = nc.declare_dram_parameter("xt", [D, S], IN_DT, isOutput=False)
    wall = nc.declare_dram_parameter("wall", [D, 384], IN_DT, isOutput=False)
    wvt = nc.declare_dram_parameter("wvt", [D, 192], IN_DT, isOutput=False)
    wproj = nc.declare_dram_parameter("wproj", [HPC * HD, D], IN_DT, isOutput=False)
    csc = nc.declare_dram_parameter("csc", [128, S], IN_DT, isOutput=False)
    css = nc.declare_dram_parameter("css", [128, S], IN_DT, isOutput=False)
    swp = nc.declare_dram_parameter("swp", [128, 128], IN_DT, isOutput=False)
    iden = nc.declare_dram_parameter("iden", [HD, HD], IN_DT, isOutput=False)
    oc3 = nc.declare_dram_parameter("oc3", [1, HPC * 128], mybir.dt.float32r,
                                    isOutput=False)
    tril = nc.declare_dram_parameter("tril", [128, 4 * 512], IN_DT,
                                     isOutput=False)
    out_d = nc.declare_dram_parameter("out", [S, D], F32, isOutput=True)

    with TileContext(nc) as tc:
        with ExitStack() as ctx:
            const = ctx.enter_context(tc.tile_pool(name="const", bufs=1))
            big = ctx.enter_context(tc.tile_pool(name="big", bufs=10))
            wpool = ctx.enter_context(tc.tile_pool(name="wpool", bufs=3))
            tpool = ctx.enter_context(tc.tile_pool(name="tpool", bufs=2))
            npool = ctx.enter_context(tc.tile_pool(name="npool", bufs=2))
            ps_a = ctx.enter_context(tc.tile_pool(name="ps_a", bufs=2, space="PSUM"))
            ps_s = ctx.enter_context(tc.tile_pool(name="ps_s", bufs=3, space="PSUM"))
            ps_y = ctx.enter_context(tc.tile_pool(name="ps_y", bufs=2, space="PSUM"))
            ps_r = ctx.enter_context(tc.tile_pool(name="ps_r", bufs=1, space="PSUM"))

            # ---- constants / inputs ----
            XT = []
            for kc in range(6):
                t = big.tile([128, S], IN_DT, tag="big", name=f"xt{kc}")
                nc.sync.dma_start(out=t, in_=x_t[kc * 128:(kc + 1) * 128, :])
                XT.append(t)
            WA = []
            for kc in range(6):
                t = const.tile([128, 384], IN_DT, tag=f"wa{kc}", name=f"wa{kc}")
                nc.sync.dma_start(out=t, in_=wall[kc * 128:(kc + 1) * 128, :])
                WA.append(t)
            WV = []
            for kc in range(6):
                t = const.tile([128, 192], IN_DT, tag=f"wv{kc}", name=f"wv{kc}")
                nc.sync.dma_start(out=t, in_=wvt[kc * 128:(kc + 1) * 128, :])
                WV.append(t)
            WP01 = const.tile([128, D], IN_DT, tag="wp01")
            nc.sync.dma_start(out=WP01, in_=wproj[0:128, :])
            WP2 = const.tile([64, D], IN_DT, tag="wp2")
            nc.sync.dma_start(out=WP2, in_=wproj[128:192, :])
            CSC = const.tile([128, S], IN_DT, tag="csc")
            nc.sync.dma_start(out=CSC, in_=csc[:, :])
            CSS = const.tile([128, S], IN_DT, tag="css")
            nc.sync.dma_start(out=CSS, in_=css[:, :])
            SWP = const.tile([128, 128], IN_DT, tag="swp")
            nc.sync.dma_start(out=SWP, in_=swp[:, :])
            ID64 = const.tile([HD, HD], IN_DT, tag="iden")
            nc.sync.dma_start(out=ID64, in_=iden[:, :])
            ONESF = const.tile([128, HD], F32, tag="onesf")
            nc.vector.memset(ONESF, 1.0)
            # denominator gather/broadcast constants:
            # OC3[64, h*128 + 32h] = 1 — scatters head h's denominator row
            # (PSUM partition 64) to partition 32h of the gather matmul out.
            F32R = mybir.dt.float32r
            OC3 = const.tile([65, HPC * 128], F32R, tag="oc3")
            nc.sync.dma_start(out=OC3[64:65, :], in_=oc3[:, :])
            TRIL = const.tile([128, 4 * 512], IN_DT, tag="tril")
            nc.sync.dma_start(out=TRIL, in_=tril[:, :])
            # ONR3 rows {0,32,64} = 1 — lhsT for the reciprocal broadcast
            ONR3 = const.tile([65, HD], IN_DT, tag="onr3")
            for h in range(HPC):
                nc.vector.memset(ONR3[32 * h:32 * h + 1, :], 1.0)

            # ---- QKV wall: 3 chunks of q/k rows ----
            C1 = big.tile([128, S], IN_DT, tag="big", name="c1")
            C2 = big.tile([128, S], IN_DT, tag="big", name="c2")
            C3 = big.tile([128, S], IN_DT, tag="big", name="c3")
            RAW = [C1, C2, C3]
            for m in range(3):
                for nt in range(NT):
                    ns = slice(nt * 512, (nt + 1) * 512)
                    ps = ps_a.tile([128, 512], F32, tag="ps_a")
                    for kc in range(6):
                        nc.tensor.matmul(
                            ps,
                            lhsT=WA[kc][:, m * 128:(m + 1) * 128],
                            rhs=XT[kc][:, ns],
                            start=(kc == 0),
                            stop=(kc == 5),
                        )
                    nc.scalar.copy(out=RAW[m][:, ns], in_=ps)

            # ---- vall: v^T per (head, t-chunk) + one-hot denominator cols ----
            vall = big.tile([128, HPC * TC * VW], IN_DT, tag="big", name="vall")
            v4 = vall.rearrange("p (h t c) -> p h t c", h=HPC, c=VW)
            nc.vector.memset(v4[:, :, :, 64:VW], 0.0)
            for h in range(HPC):
                nc.vector.tensor_copy(out=v4[:, h, :, 64], in_=ONESF[:, 0:TC])
            for tcb in range(TC):
                pv = ps_a.tile([128, 512], F32, tag="ps_a")
                for kc in range(6):
                    nc.tensor.matmul(
                        pv[:, 0:192],
                        lhsT=XT[kc][:, tcb * 128:(tcb + 1) * 128],
                        rhs=WV[kc],
                        start=(kc == 0),
                        stop=(kc == 5),
                    )
                # scatter [128, 3, 64] psum -> the 3 heads' v slots
                nc.vector.tensor_copy(
                    out=v4[:, :, tcb, 0:64],
                    in_=pv[:, 0:192].rearrange("p (h c) -> p h c", h=HPC),
                )

            def vsl(h, tcb):
                return vall[:, (h * TC + tcb) * VW:(h * TC + tcb) * VW + 65]

            # ---- RoPE: out = raw*cos + swap(raw)*sin' (sign baked in css) ----
            QF = big.tile([128, S], IN_DT, tag="big", name="qf")
            KF = big.tile([128, S], IN_DT, tag="big", name="kf")
            G3 = big.tile([128, S], IN_DT, tag="big", name="g3")

            def rope(raw, out):
                for nt in range(NT):
                    ns = slice(nt * 512, (nt + 1) * 512)
                    sw = ps_a.tile([128, 512], F32, tag="ps_a")
                    nc.tensor.matmul(
                        sw, lhsT=SWP, rhs=raw[:, ns], start=True, stop=True
                    )
                    swb = tpool.tile([128, 512], IN_DT, tag="swb")
                    nc.scalar.copy(out=swb, in_=sw)
                    t1 = tpool.tile([128, 512], IN_DT, tag="t1")
                    t2 = tpool.tile([128, 512], IN_DT, tag="t2")
                    nc.vector.tensor_mul(t1, raw[:, ns], CSC[:, ns])
                    nc.vector.tensor_mul(t2, swb, CSS[:, ns])
                    nc.vector.tensor_add(out[:, ns], t1, t2)

            rope(C1, QF)     # q_h0, q_h1
            rope(C2, KF)     # k_h0, k_h1
            rope(C3, G3)     # q_h2 | k_h2

            # relocate roped q2 to partitions 64:128 so the h2 score matmul's
            # lhsT/rhs share a base partition (hardware requirement)
            Q2R = big.tile([128, S], IN_DT, tag="big", name="q2r")
            for nt in range(NT):
                ns = slice(nt * 512, (nt + 1) * 512)
                rq = ps_a.tile([128, 512], F32, tag="ps_a")
                nc.tensor.matmul(rq[64:128, :], lhsT=ID64, rhs=G3[0:64, ns],
                                 start=True, stop=True)
                nc.scalar.copy(out=Q2R[64:128, ns], in_=rq[64:128, :])

            QSRC = [QF[0:64, :], QF[64:128, :], Q2R[64:128, :]]
            KSRC = [KF[0:64, :], KF[64:128, :], G3[64:128, :]]

            YT01 = big.tile([128, S], IN_DT, tag="big", name="yt01")
            YT2 = big.tile([64, S], IN_DT, tag="big", name="yt2")

            # ---- attention: si outer so the 3 heads' denominators batch ----
            for si in range(NT):
                ss = slice(si * 512, (si + 1) * 512)
                ntc = 4 * (si + 1)
                rep = ps_r.tile([128, 512], F32, tag="ps_r")
                DG = ps_a.tile([128, 512], F32, tag="ps_a")
                for h in range(HPC):
                    q, k = QSRC[h], KSRC[h]
                    yps = ps_y.tile([65, 512], F32, tag="ps_y")
                    for tcb in range(ntc):
                        sc = ps_s.tile([128, 512], F32, tag="ps_s")
                        nc.tensor.matmul(
                            sc,
                            lhsT=k[:, tcb * 128:(tcb + 1) * 128],
                            rhs=q[:, ss],
                            start=True,
                            stop=True,
                        )
                        wt = wpool.tile([128, 512], IN_DT, tag="wt")
                        nc.scalar.activation(
                            out=wt, in_=sc,
                            func=getattr(mybir.ActivationFunctionType, ERF_FUNC_NAME),
                            scale=0.125,
                        )
                        if tcb >= 4 * si:
                            # diagonal band: (erf+1) * causal mask, fused
                            j = tcb - 4 * si
                            nc.vector.scalar_tensor_tensor(
                                out=wt, in0=wt, scalar=1.0,
                                in1=TRIL[:, j * 512:(j + 1) * 512],
                                op0=mybir.AluOpType.add,
                                op1=mybir.AluOpType.mult,
                            )
                        else:
                            nc.vector.tensor_scalar_add(wt, wt, 1.0)
                        nc.tensor.matmul(
                            yps,
                            lhsT=vsl(h, tcb),
                            rhs=wt,
                            start=(tcb == 0),
                            stop=(tcb == ntc - 1),
                        )
                    # stash unnormalized yT; scatter this head's denominator
                    # row (PSUM partition 64) to partition 32h of DG
                    dst = (YT01[0:64, ss] if h == 0 else
                           YT01[64:128, ss] if h == 1 else YT2[:, ss])
                    nc.vector.tensor_copy(out=dst, in_=yps[0:64, :])
                    SD = npool.tile([65, 512], F32R, tag="sd")
                    nc.scalar.copy(out=SD[64:65, :], in_=yps[64:65, :])
                    nc.tensor.matmul(
                        DG, lhsT=OC3[64:65, h * 128:(h + 1) * 128],
                        rhs=SD[64:65, :],
                        start=(h == 0), stop=(h == HPC - 1),
                    )

                # one batched reciprocal for the 3 heads of this si block,
                # broadcast to all 64 dims via K=1 matmuls on rows {0,32,64}
                DGS = npool.tile([128, 512], F32, tag="dgs")
                nc.scalar.copy(out=DGS, in_=DG)
                RC = npool.tile([128, 512], F32, tag="rc")
                nc.vector.reciprocal(RC, DGS)
                RCB = npool.tile([128, 512], IN_DT, tag="rcb")
                nc.vector.tensor_copy(RCB, RC)
                rep2 = ps_y.tile([65, 512], F32, tag="ps_y")
                for h in range(HPC):
                    rdst = (rep[0:64, :] if h == 0 else
                            rep[64:128, :] if h == 1 else rep2[0:64, :])
                    nc.tensor.matmul(
                        rdst, lhsT=ONR3[32 * h:32 * h + 1, :],
                        rhs=RCB[32 * h:32 * h + 1, :],
                        start=True, stop=True,
                    )
                rsb = npool.tile([128, 512], IN_DT, tag="rsb")
                nc.scalar.copy(out=rsb, in_=rep)
                rsb2 = npool.tile([64, 512], IN_DT, tag="rsb2")
                nc.scalar.copy(out=rsb2, in_=rep2[0:64, :])
                nc.vector.tensor_mul(YT01[:, ss], YT01[:, ss], rsb)
                nc.vector.tensor_mul(YT2[:, ss], YT2[:, ss], rsb2)

            # ---- output projection (partial over this core's heads) ----
            for sci in range(TC):
                scs = slice(sci * 128, (sci + 1) * 128)
                po1 = ps_a.tile([128, 512], F32, tag="ps_a")
                po2 = ps_a.tile([128, 512], F32, tag="ps_a")
                nc.tensor.matmul(po1, lhsT=YT01[:, scs], rhs=WP01[:, 0:512],
                                 start=True, stop=False)
                nc.tensor.matmul(po1, lhsT=YT2[:, scs], rhs=WP2[:, 0:512],
                                 start=False, stop=True)
                nc.tensor.matmul(po2[:, 0:256], lhsT=YT01[:, scs],
                                 rhs=WP01[:, 512:768], start=True, stop=False)
                nc.tensor.matmul(po2[:, 0:256], lhsT=YT2[:, scs],
                                 rhs=WP2[:, 512:768], start=False, stop=True)
                ost = tpool.tile([128, D], F32, tag="ost", bufs=3)
                if sci % 2 == 0:
                    nc.scalar.copy(out=ost[:, 0:512], in_=po1)
                    nc.vector.tensor_copy(out=ost[:, 512:768], in_=po2[:, 0:256])
                else:
                    nc.vector.tensor_copy(out=ost[:, 0:512], in_=po1)
                    nc.scalar.copy(out=ost[:, 512:768], in_=po2[:, 0:256])
                nc.sync.dma_start(out=out_d[scs, :], in_=ost)

    return nc


_PROGRAM = None


def _get_program() -> bass.Bass:
    global _PROGRAM
    if _PROGRAM is None:
        _PROGRAM = build_program()
        _split_multi_waits(_PROGRAM)
    return _PROGRAM


def _np_indt(arr):
    return np.ascontiguousarray(arr).astype(mybir.dt.np(IN_DT))


def make_in_maps(x, Wq, Wk, Wv, Wproj):
    x = np.asarray(x, dtype=np.float32)
    Wq = np.asarray(Wq, dtype=np.float32)
    Wk = np.asarray(Wk, dtype=np.float32)
    Wv = np.asarray(Wv, dtype=np.float32)
    Wproj = np.asarray(Wproj, dtype=np.float32)

    half = HD // 2
    j = np.arange(half, dtype=np.float64)
    freq = 1.0 / (10000.0 ** (j / half))
    ang = np.arange(S, dtype=np.float64)[None, :] * freq[:, None]   # [32, S]
    cosT = np.cos(ang).astype(np.float32)
    sinT = np.sin(ang).astype(np.float32)
    csc = np.tile(np.vstack([cosT, cosT]), (2, 1))                  # [128, S]
    css = np.tile(np.vstack([-sinT, sinT]), (2, 1))

    swp = np.zeros((128, 128), dtype=np.float32)
    for blk in range(2):
        for jj in range(half):
            swp[blk * 64 + jj, blk * 64 + half + jj] = 1.0
            swp[blk * 64 + half + jj, blk * 64 + jj] = 1.0

    oc3m = np.zeros((1, HPC * 128), dtype=np.float32)
    for h in range(HPC):
        oc3m[0, h * 128 + 32 * h] = 1.0

    trilm = np.zeros((128, 4 * 512), dtype=np.float32)
    tt = np.arange(128)[:, None]
    sl = np.arange(512)[None, :]
    for jj in range(4):
        trilm[:, jj * 512:(jj + 1) * 512] = (tt <= sl - 128 * jj)

    perm = np.concatenate([np.arange(0, HD, 2), np.arange(1, HD, 2)])

    in_maps = []
    for c in range(NCORES):
        b = c // 4
        hs = [(c % 4) * HPC + i for i in range(HPC)]
        rq = [Wq[h * HD:(h + 1) * HD][perm, :] for h in hs]
        rk = [Wk[h * HD:(h + 1) * HD][perm, :] for h in hs]
        cols = np.concatenate(
            [rq[0], rq[1], rk[0], rk[1], rq[2], rk[2]], axis=0
        )                                                           # [384, D]
        wallm = np.ascontiguousarray(cols.T)                        # [D, 384]
        wvtm = np.ascontiguousarray(
            np.concatenate([Wv[h * HD:(h + 1) * HD] for h in hs], axis=0).T
        )                                                           # [D, 192]
        dims = np.concatenate([np.arange(h * HD, (h + 1) * HD) for h in hs])
        wproj_t = np.ascontiguousarray(Wproj[:, dims].T)            # [192, D]
        in_maps.append({
            "xt": _np_indt(x[b].T),
            "wall": _np_indt(wallm),
            "wvt": _np_indt(wvtm),
            "wproj": _np_indt(wproj_t),
            "csc": _np_indt(csc),
            "css": _np_indt(css),
            "swp": _np_indt(swp),
            "iden": _np_indt(np.eye(HD, dtype=np.float32)),
            "oc3": oc3m,
            "tril": _np_indt(trilm),
        })
    return in_maps


def kernel(x, Wq, Wk, Wv, Wproj):
    global LAST_EXEC_NS, LAST_RESULTS
    nc = _get_program()
    in_maps = make_in_maps(x, Wq, Wk, Wv, Wproj)
    trace = os.environ.get("KERNEL_TRACE", "0") == "1"
    res = run_bass_kernel_spmd(nc, in_maps, list(range(NCORES)), trace=trace)
    LAST_EXEC_NS = res.exec_time_ns
    LAST_RESULTS = res
    outs = [np.asarray(r["out"], dtype=np.float32) for r in res.results]
    out = np.empty((2, S, D), dtype=np.float32)
    out[0] = outs[0] + outs[1] + outs[2] + outs[3]
    out[1] = outs[4] + outs[5] + outs[6] + outs[7]
    return out


# revision 36
# speedup vs baseline: 1.2968x; 1.0394x over previous
"""Trainium2 Bass kernel for nn_CausalSelfAttention (erf-kernel attention).

Sharding: 8 cores = 2 batches x 4 core-groups; each core handles one batch
and 3 of the 12 heads (data-parallel over batch, head-parallel within batch).
Each core computes its 3 heads' full attention plus its partial output
projection; the host sums the 4 partials per batch.

Device-side layout strategy (per core), all matmul inputs in bf16:
  - x arrives pre-transposed from host: xT [768, 2048] bf16.
  - Q/K weight "wall" [768, 384]: chunks C1=[q0|q1], C2=[k0|k1], C3=[q2|k2],
    each head's rows rope-permuted ([even dims | odd dims]) so RoPE operates
    on contiguous 32-partition blocks.
  - v^T computed directly: per 128-t-chunk, psum[128,192] = xT[:,tch].T@WvT,
    scattered into vall [128, 3*16*68]: per (head, chunk) 68 cols =
    [v^T (64) | one-hot ones column at 64+h | pad].  The ones column makes
    the AV matmul emit that head's softmax denominator at psum row 64+h, a
    distinct partition per head so denominators batch across heads.
  - RoPE: partner swap via PE permutation matmul, cos/sin multiplies on DVE
    in bf16 (2x perf mode), swap output staged through ACT copy.
  - Scores transposed: sT[t,s] = kT.T @ qT per (128t x 512s) causal tile,
    erf(0.125*x) on ACT -> bf16, +1 on DVE (4x mode), diagonal band masked
    with affine_select on Pool.
  - AV: yT[d,s] accumulated in PSUM over t-chunks, M=68 (64 dims + one-hot
    denominator rows).  Unnormalized yT copied to SBUF; denominators for the
    3 heads land on partitions 64..66 and are reciprocal-approximated in one
    batched DVE op per si, broadcast via a K=3 matmul with one-hot E3.
  - Projection: heads K-stacked (YT01 [128,S] + YT2 [64,S]): 2 accumulating
    matmuls per output half instead of 3.
"""

import os
import sys
from contextlib import ExitStack

import numpy as np

for _p in ("/opt/trn_rl_repo",):
    if _p not in sys.path:
        sys.path.insert(0, _p)

import concourse.bass as bass
import concourse.mybir as mybir
from concourse.bass_utils import run_bass_kernel_spmd
from concourse.tile import TileContext

S = 2048          # sequence length per batch
D = 768           # model dim
HD = 64           # head dim
HPC = 3           # heads per core
NCORES = 8
F32 = mybir.dt.float32
NT = S // 512     # 4 free-dim tiles of 512
TC = S // 128     # 16 t-chunks of 128
VW = 66           # vall chunk width: 64 v dims + ones col + pad

DTYPE_NAME = os.environ.get("KERNEL_DTYPE", "bf16")
IN_DT = {
    "fp32": mybir.dt.float32,
    "f32r": mybir.dt.float32r,
    "bf16": mybir.dt.bfloat16,
}[DTYPE_NAME]
# CoreSim doesn't implement Erf; dev-only switch to validate logic in sim.
ERF_FUNC_NAME = "Tanh" if os.environ.get("KERNEL_SIM_TANH", "0") == "1" else "Erf"

LAST_EXEC_NS = None
LAST_RESULTS = None


def _split_multi_waits(nc: bass.Bass) -> None:
    """This walrus build rejects instructions carrying more than one sync
    wait (codegen 'Too many sync wait commands', hit by the Tile kernel-tail
    drain).  Hoist all but the last wait of any multi-wait instruction onto
    single-wait Drain instructions inserted just before it on the same
    engine — semantically identical, one wait per instruction."""
    for f in nc.m.functions:
        for b in f.blocks:
            new_insts = []
            changed = False
            for inst in b.instructions:
                si = inst.sync_info
                waits = list(si.on_wait) if si is not None and si.on_wait else []
                if len(waits) > 1:
                    changed = True
                    for n, w in enumerate(waits[:-1]):
                        d = mybir.InstDrain(
                            name=f"{inst.name}-wsplit{n}",
                            engine=inst.engine,
                            ins=[],
                            outs=[],
                            sync_info=mybir.SyncInfo(on_wait=[w], on_update=[]),
                        )
                        new_insts.append(d)
                    si.on_wait = [waits[-1]]
                new_insts.append(inst)
            if changed:
                b.instructions[:] = new_insts


def build_program() -> bass.Bass:
    nc = bass.Bass(target_bir_lowering=False, debug=False)

    x_t = nc.declare_dram_parameter("xt", [D, S], IN_DT, isOutput=False)
    wall = nc.declare_dram_parameter("wall", [D, 384], IN_DT, isOutput=False)
    wvt = nc.declare_dram_parameter("wvt", [D, 192], IN_DT, isOutput=False)
    wproj = nc.declare_dram_parameter("wproj", [HPC * HD, D], IN_DT, isOutput=False)
    csc = nc.declare_dram_parameter("csc", [128, S], IN_DT, isOutput=False)
    css = nc.declare_dram_parameter("css", [128, S], IN_DT, isOutput=False)
    swp = nc.declare_dram_parameter("swp", [128, 128], IN_DT, isOutput=False)
    iden = nc.declare_dram_parameter("iden", [HD, HD], IN_DT, isOutput=False)
    oc3 = nc.declare_dram_parameter("oc3", [1, HPC * 128], mybir.dt.float32r,
                                    isOutput=False)
    tril = nc.declare_dram_parameter("tril", [128, 4 * 512], IN_DT,
                                     isOutput=False)
    out_d = nc.declare_dram_parameter("out", [S, D], F32, isOutput=True)

    with TileContext(nc) as tc:
        with ExitStack() as ctx:
            const = ctx.enter_context(tc.tile_pool(name="const", bufs=1))
            big = ctx.enter_context(tc.tile_pool(name="big", bufs=10))
            wpool = ctx.enter_context(tc.tile_pool(name="wpool", bufs=3))
            tpool = ctx.enter_context(tc.tile_pool(name="tpool", bufs=2))
            npool = ctx.enter_context(tc.tile_pool(name="npool", bufs=2))
            ps_a = ctx.enter_context(tc.tile_pool(name="ps_a", bufs=1, space="PSUM"))
            ps_s = ctx.enter_context(tc.tile_pool(name="ps_s", bufs=2, space="PSUM"))
            ps_y = ctx.enter_context(tc.tile_pool(name="ps_y", bufs=2, space="PSUM"))
            ps_r = ctx.enter_context(tc.tile_pool(name="ps_r", bufs=1, space="PSUM"))

            # ---- constants / inputs ----
            XT = []
            for kc in range(6):
                t = big.tile([128, S], IN_DT, tag="big", name=f"xt{kc}")
                nc.sync.dma_start(out=t, in_=x_t[kc * 128:(kc + 1) * 128, :])
                XT.append(t)
            WA = []
            for kc in range(6):
                t = const.tile([128, 384], IN_DT, tag=f"wa{kc}", name=f"wa{kc}")
                nc.sync.dma_start(out=t, in_=wall[kc * 128:(kc + 1) * 128, :])
                WA.append(t)
            WV = []
            for kc in range(6):
                t = const.tile([128, 192], IN_DT, tag=f"wv{kc}", name=f"wv{kc}")
                nc.sync.dma_start(out=t, in_=wvt[kc * 128:(kc + 1) * 128, :])
                WV.append(t)
            WP01 = const.tile([128, D], IN_DT, tag="wp01")
            nc.sync.dma_start(out=WP01, in_=wproj[0:128, :])
            WP2 = const.tile([64, D], IN_DT, tag="wp2")
            nc.sync.dma_start(out=WP2, in_=wproj[128:192, :])
            CSC = const.tile([128, S], IN_DT, tag="csc")
            nc.sync.dma_start(out=CSC, in_=csc[:, :])
            CSS = const.tile([128, S], IN_DT, tag="css")
            nc.sync.dma_start(out=CSS, in_=css[:, :])
            SWP = const.tile([128, 128], IN_DT, tag="swp")
            nc.sync.dma_start(out=SWP, in_=swp[:, :])
            ID64 = const.tile([HD, HD], IN_DT, tag="iden")
            nc.sync.dma_start(out=ID64, in_=iden[:, :])
            ONESF = const.tile([128, HD], F32, tag="onesf")
            nc.vector.memset(ONESF, 1.0)
            # denominator gather/broadcast constants:
            # OC3[64, h*128 + 32h] = 1 — scatters head h's denominator row
            # (PSUM partition 64) to partition 32h of the gather matmul out.
            F32R = mybir.dt.float32r
            OC3 = const.tile([65, HPC * 128], F32R, tag="oc3")
            nc.sync.dma_start(out=OC3[64:65, :], in_=oc3[:, :])
            TRIL = const.tile([128, 4 * 512], IN_DT, tag="tril")
            nc.sync.dma_start(out=TRIL, in_=tril[:, :])
            # ONR3 rows {0,32,64} = 1 — lhsT for the reciprocal broadcast
            ONR3 = const.tile([65, HD], IN_DT, tag="onr3")
            for h in range(HPC):
                nc.vector.memset(ONR3[32 * h:32 * h + 1, :], 1.0)

            # ---- QKV wall: 3 chunks of q/k rows ----
            C1 = big.tile([128, S], IN_DT, tag="big", name="c1")
            C2 = big.tile([128, S], IN_DT, tag="big", name="c2")
            C3 = big.tile([128, S], IN_DT, tag="big", name="c3")
            RAW = [C1, C2, C3]
            for m in range(3):
                for nt in range(NT):
                    ns = slice(nt * 512, (nt + 1) * 512)
                    ps = ps_s.tile([128, 1024], F32, tag="ps_s")
                    for kc in range(6):
                        nc.tensor.matmul(
                            ps[:, 0:512],
                            lhsT=WA[kc][:, m * 128:(m + 1) * 128],
                            rhs=XT[kc][:, ns],
                            start=(kc == 0),
                            stop=(kc == 5),
                        )
                    nc.scalar.copy(out=RAW[m][:, ns], in_=ps[:, 0:512])

            # ---- vall: v^T per (head, t-chunk) + one-hot denominator cols ----
            vall = big.tile([128, HPC * TC * VW], IN_DT, tag="big", name="vall")
            v4 = vall.rearrange("p (h t c) -> p h t c", h=HPC, c=VW)
            nc.vector.memset(v4[:, :, :, 64:VW], 0.0)
            for h in range(HPC):
                nc.vector.tensor_copy(out=v4[:, h, :, 64], in_=ONESF[:, 0:TC])
            for tcb in range(TC):
                pv = ps_s.tile([128, 1024], F32, tag="ps_s")
                for kc in range(6):
                    nc.tensor.matmul(
                        pv[:, 0:192],
                        lhsT=XT[kc][:, tcb * 128:(tcb + 1) * 128],
                        rhs=WV[kc],
                        start=(kc == 0),
                        stop=(kc == 5),
                    )
                # scatter [128, 3, 64] psum -> the 3 heads' v slots
                nc.vector.tensor_copy(
                    out=v4[:, :, tcb, 0:64],
                    in_=pv[:, 0:192].rearrange("p (h c) -> p h c", h=HPC),
                )

            def vsl(h, tcb):
                return vall[:, (h * TC + tcb) * VW:(h * TC + tcb) * VW + 65]

            # ---- RoPE: out = raw*cos + swap(raw)*sin' (sign baked in css) ----
            QF = big.tile([128, S], IN_DT, tag="big", name="qf")
            KF = big.tile([128, S], IN_DT, tag="big", name="kf")
            G3 = big.tile([128, S], IN_DT, tag="big", name="g3")

            def rope(raw, out):
                for nt in range(NT):
                    ns = slice(nt * 512, (nt + 1) * 512)
                    swt = ps_s.tile([128, 1024], F32, tag="ps_s")
                    sw = swt[:, 0:512]
                    nc.tensor.matmul(
                        sw, lhsT=SWP, rhs=raw[:, ns], start=True, stop=True
                    )
                    swb = tpool.tile([128, 512], IN_DT, tag="swb")
                    nc.scalar.copy(out=swb, in_=sw)
                    t1 = tpool.tile([128, 512], IN_DT, tag="t1")
                    t2 = tpool.tile([128, 512], IN_DT, tag="t2")
                    nc.vector.tensor_mul(t1, raw[:, ns], CSC[:, ns])
                    nc.vector.tensor_mul(t2, swb, CSS[:, ns])
                    nc.vector.tensor_add(out[:, ns], t1, t2)

            rope(C1, QF)     # q_h0, q_h1
            rope(C2, KF)     # k_h0, k_h1
            rope(C3, G3)     # q_h2 | k_h2

            # relocate roped q2 to partitions 64:128 so the h2 score matmul's
            # lhsT/rhs share a base partition (hardware requirement)
            Q2R = big.tile([128, S], IN_DT, tag="big", name="q2r")
            for nt in range(NT):
                ns = slice(nt * 512, (nt + 1) * 512)
                rq = ps_s.tile([128, 1024], F32, tag="ps_s")
                nc.tensor.matmul(rq[64:128, 0:512], lhsT=ID64, rhs=G3[0:64, ns],
                                 start=True, stop=True)
                nc.scalar.copy(out=Q2R[64:128, ns], in_=rq[64:128, 0:512])

            QSRC = [QF[0:64, :], QF[64:128, :], Q2R[64:128, :]]
            KSRC = [KF[0:64, :], KF[64:128, :], G3[64:128, :]]

            YT01 = big.tile([128, S], IN_DT, tag="big", name="yt01")
            YT2 = big.tile([64, S], IN_DT, tag="big", name="yt2")

            # ---- attention: si outer so the 3 heads' denominators batch ----
            for si in range(NT):
                ss = slice(si * 512, (si + 1) * 512)
                ntc = 4 * (si + 1)
                rep = ps_r.tile([128, 512], F32, tag="ps_r")
                DG = ps_a.tile([128, 512], F32, tag="ps_a")
                for h in range(HPC):
                    q, k = QSRC[h], KSRC[h]
                    yps = ps_y.tile([65, 512], F32, tag="ps_y")
                    npair = ntc // 2
                    for p in range(npair):
                        tc0, tc1 = 2 * p, 2 * p + 1
                        sc = ps_s.tile([128, 1024], F32, tag="ps_s")
                        nc.tensor.matmul(
                            sc[:, 0:512],
                            lhsT=k[:, tc0 * 128:(tc0 + 1) * 128],
                            rhs=q[:, ss], start=True, stop=True,
                        )
                        nc.tensor.matmul(
                            sc[:, 512:1024],
                            lhsT=k[:, tc1 * 128:(tc1 + 1) * 128],
                            rhs=q[:, ss], start=True, stop=True,
                        )
                        wt = wpool.tile([128, 1024], IN_DT, tag="wt")
                        nc.scalar.activation(
                            out=wt, in_=sc,
                            func=getattr(mybir.ActivationFunctionType, ERF_FUNC_NAME),
                            scale=0.125,
                        )
                        nc.vector.tensor_scalar_add(wt, wt, 1.0)
                        if tc0 >= 4 * si:
                            # diagonal pair: * causal mask (bf16 2x TT)
                            j = tc0 - 4 * si
                            nc.vector.tensor_mul(
                                wt, wt, TRIL[:, j * 512:(j + 2) * 512]
                            )
                        nc.tensor.matmul(
                            yps, lhsT=vsl(h, tc0), rhs=wt[:, 0:512],
                            start=(p == 0), stop=False,
                        )
                        nc.tensor.matmul(
                            yps, lhsT=vsl(h, tc1), rhs=wt[:, 512:1024],
                            start=False, stop=(p == npair - 1),
                        )
                    # stash unnormalized yT; scatter this head's denominator
                    # row (PSUM partition 64) to partition 32h of DG
                    dst = (YT01[0:64, ss] if h == 0 else
                           YT01[64:128, ss] if h == 1 else YT2[:, ss])
                    nc.vector.tensor_copy(out=dst, in_=yps[0:64, :])
                    SD = npool.tile([65, 512], F32R, tag="sd")
                    nc.scalar.copy(out=SD[64:65, :], in_=yps[64:65, :])
                    nc.tensor.matmul(
                        DG, lhsT=OC3[64:65, h * 128:(h + 1) * 128],
                        rhs=SD[64:65, :],
                        start=(h == 0), stop=(h == HPC - 1),
                    )

                # one batched reciprocal for the 3 heads of this si block,
                # broadcast to all 64 dims via K=1 matmuls on rows {0,32,64}
                DGS = npool.tile([128, 512], F32, tag="dgs")
                nc.scalar.copy(out=DGS, in_=DG)
                RC = npool.tile([128, 512], F32, tag="rc")
                nc.vector.reciprocal(RC, DGS)
                RCB = npool.tile([128, 512], IN_DT, tag="rcb")
                nc.vector.tensor_copy(RCB, RC)
                rep2 = ps_y.tile([65, 512], F32, tag="ps_y")
                for h in range(HPC):
                    rdst = (rep[0:64, :] if h == 0 else
                            rep[64:128, :] if h == 1 else rep2[0:64, :])
                    nc.tensor.matmul(
                        rdst, lhsT=ONR3[32 * h:32 * h + 1, :],
                        rhs=RCB[32 * h:32 * h + 1, :],
                        start=True, stop=True,
                    )
                rsb = npool.tile([128, 512], IN_DT, tag="rsb")
                nc.scalar.copy(out=rsb, in_=rep)
                rsb2 = npool.tile([64, 512], IN_DT, tag="rsb2")
                nc.scalar.copy(out=rsb2, in_=rep2[0:64, :])
                nc.vector.tensor_mul(YT01[:, ss], YT01[:, ss], rsb)
                nc.vector.tensor_mul(YT2[:, ss], YT2[:, ss], rsb2)

            # ---- output projection (partial over this core's heads) ----
            for sci in range(TC):
                scs = slice(sci * 128, (sci + 1) * 128)
                po = ps_s.tile([128, 1024], F32, tag="ps_s")
                nc.tensor.matmul(po[:, 0:512], lhsT=YT01[:, scs],
                                 rhs=WP01[:, 0:512], start=True, stop=False)
                nc.tensor.matmul(po[:, 0:512], lhsT=YT2[:, scs],
                                 rhs=WP2[:, 0:512], start=False, stop=True)
                nc.tensor.matmul(po[:, 512:768], lhsT=YT01[:, scs],
                                 rhs=WP01[:, 512:768], start=True, stop=False)
                nc.tensor.matmul(po[:, 512:768], lhsT=YT2[:, scs],
                                 rhs=WP2[:, 512:768], start=False, stop=True)
                ost = tpool.tile([128, D], F32, tag="ost", bufs=3)
                if sci % 2 == 0:
                    nc.scalar.copy(out=ost[:, 0:512], in_=po[:, 0:512])
                    nc.vector.tensor_copy(out=ost[:, 512:768], in_=po[:, 512:768])
                else:
                    nc.vector.tensor_copy(out=ost[:, 0:512], in_=po[:, 0:512])
                    nc.scalar.copy(out=ost[:, 512:768], in_=po[:, 512:768])
                nc.sync.dma_start(out=out_d[scs, :], in_=ost)

    return nc


_PROGRAM = None


def _get_program() -> bass.Bass:
    global _PROGRAM
    if _PROGRAM is None:
        _PROGRAM = build_program()
        _split_multi_waits(_PROGRAM)
    return _PROGRAM


def _np_indt(arr):
    return np.ascontiguousarray(arr).astype(mybir.dt.np(IN_DT))


def make_in_maps(x, Wq, Wk, Wv, Wproj):
    x = np.asarray(x, dtype=np.float32)
    Wq = np.asarray(Wq, dtype=np.float32)
    Wk = np.asarray(Wk, dtype=np.float32)
    Wv = np.asarray(Wv, dtype=np.float32)
    Wproj = np.asarray(Wproj, dtype=np.float32)

    half = HD // 2
    j = np.arange(half, dtype=np.float64)
    freq = 1.0 / (10000.0 ** (j / half))
    ang = np.arange(S, dtype=np.float64)[None, :] * freq[:, None]   # [32, S]
    cosT = np.cos(ang).astype(np.float32)
    sinT = np.sin(ang).astype(np.float32)
    csc = np.tile(np.vstack([cosT, cosT]), (2, 1))                  # [128, S]
    css = np.tile(np.vstack([-sinT, sinT]), (2, 1))

    swp = np.zeros((128, 128), dtype=np.float32)
    for blk in range(2):
        for jj in range(half):
            swp[blk * 64 + jj, blk * 64 + half + jj] = 1.0
            swp[blk * 64 + half + jj, blk * 64 + jj] = 1.0

    oc3m = np.zeros((1, HPC * 128), dtype=np.float32)
    for h in range(HPC):
        oc3m[0, h * 128 + 32 * h] = 1.0

    trilm = np.zeros((128, 4 * 512), dtype=np.float32)
    tt = np.arange(128)[:, None]
    sl = np.arange(512)[None, :]
    for jj in range(4):
        trilm[:, jj * 512:(jj + 1) * 512] = (tt <= sl - 128 * jj)

    perm = np.concatenate([np.arange(0, HD, 2), np.arange(1, HD, 2)])

    in_maps = []
    for c in range(NCORES):
        b = c // 4
        hs = [(c % 4) * HPC + i for i in range(HPC)]
        rq = [Wq[h * HD:(h + 1) * HD][perm, :] for h in hs]
        rk = [Wk[h * HD:(h + 1) * HD][perm, :] for h in hs]
        cols = np.concatenate(
            [rq[0], rq[1], rk[0], rk[1], rq[2], rk[2]], axis=0
        )                                                           # [384, D]
        wallm = np.ascontiguousarray(cols.T)                        # [D, 384]
        wvtm = np.ascontiguousarray(
            np.concatenate([Wv[h * HD:(h + 1) * HD] for h in hs], axis=0).T
        )                                                           # [D, 192]
        dims = np.concatenate([np.arange(h * HD, (h + 1) * HD) for h in hs])
        wproj_t = np.ascontiguousarray(Wproj[:, dims].T)            # [192, D]
        in_maps.append({
            "xt": _np_indt(x[b].T),
            "wall": _np_indt(wallm),
            "wvt": _np_indt(wvtm),
            "wproj": _np_indt(wproj_t),
            "csc": _np_indt(csc),
            "css": _np_indt(css),
            "swp": _np_indt(swp),
            "iden": _np_indt(np.eye(HD, dtype=np.float32)),
            "oc3": oc3m,
            "tril": _np_indt(trilm),
        })
    return in_maps


def kernel(x, Wq, Wk, Wv, Wproj):
    global LAST_EXEC_NS, LAST_RESULTS
    nc = _get_program()
    in_maps = make_in_maps(x, Wq, Wk, Wv, Wproj)
    trace = os.environ.get("KERNEL_TRACE", "0") == "1"
    res = run_bass_kernel_spmd(nc, in_maps, list(range(NCORES)), trace=trace)
    LAST_EXEC_NS = res.exec_time_ns
    LAST_RESULTS = res
    outs = [np.asarray(r["out"], dtype=np.float32) for r in res.results]
    out = np.empty((2, S, D), dtype=np.float32)
    out[0] = outs[0] + outs[1] + outs[2] + outs[3]
    out[1] = outs[4] + outs[5] + outs[6] + outs[7]
    return out
